# revision 3
# baseline (speedup 1.0000x reference)
"""Masked multi-head attention kernel for Trainium2 (Bass/Tile), 8-core SPMD.

Problem: BH=64 heads of S=2048, D=64 attention with a dense bool mask,
scale = 1/sqrt(1024).  Sharded 8 heads per NeuronCore (no cross-core comm).

Per-core dataflow (heads processed in pairs):
  - Q,K loaded f32, cast to bf16, PE-transposed into QT/KT slabs [d, S]
    with head A on partitions 0-63 and head B on partitions 64-127.
  - S^T[k,q] = K @ Q^T computed with row-tiled paired matmuls (head A in
    PE rows 0-63, head B in rows 64-127; they run concurrently).
  - The bool mask is applied on the PE: mask[q,k] tiles (DMA-cast u8->bf16)
    are used as the stationary operand against a -960*I identity, which
    accumulates -960*mask^T into the same PSUM tile.  After the ACT exp
    with scale=1/32 this is exp(S - 30*mask) ~= 0 for masked entries.
  - exp on the scalar engine PSUM->SBUF (bf16 out) builds the P^T slab.
  - AV: for each k-chunk, stationary [V | 1] (M=65) streams P^T, giving
    O^T (rows 0-63) and the softmax denominators l (row 64) in PSUM.
  - Epilogue: PE-transpose O^T back to natural [q, d], reciprocal of l,
    per-partition scale on the vector engine, natural DMA store.
"""

import os
import sys

sys.path.insert(0, "/opt/trn_rl_repo")

import numpy as np

import concourse.bass as bass
import concourse.mybir as mybir
import concourse.tile as tile
from concourse import bacc
from concourse.bass_utils import run_bass_kernel_spmd
from concourse.masks import make_identity

N_CORES = 8
BH, S_FULL, D = 64, 2048, 64
H_PER_CORE = BH // N_CORES  # 8
P = 128  # SBUF/PSUM partitions
KCH = 128  # k-chunk (S^T partition tile)
SCALE = 1.0 / 32.0  # 1/sqrt(1024) per the module spec
NEGC = -960.0  # -960/32 = -30 after the ACT scale -> exp ~ 9e-14


def build_attention(tc, o_ap, q_ap, k_ap, v_ap, m_ap, H, S, qch):
    nc = tc.nc
    dt = mybir.dt
    n_pairs = H // 2
    n_kch = S // KCH
    n_qt = S // P
    n_qch = S // qch
    QS = qch // P  # q-subtiles per chunk

    with (
        tc.tile_pool(name="const", bufs=1) as constp,
        tc.tile_pool(name="stage", bufs=8) as stagep,
        tc.tile_pool(name="qkslab", bufs=2) as qkp,
        tc.tile_pool(name="vp", bufs=4 * n_kch) as vpool,
        tc.tile_pool(name="maskp", bufs=4 * QS) as maskp,
        tc.tile_pool(name="ptp", bufs=4 * n_kch) as ptp,
        tc.tile_pool(name="op", bufs=4) as opool,
        tc.tile_pool(name="smallp", bufs=8) as smallp,
        tc.tile_pool(name="ps_s", bufs=4, space="PSUM") as ps_s,
        tc.tile_pool(name="ps_t", bufs=2, space="PSUM") as ps_t,
        tc.tile_pool(name="ps_o", bufs=2, space="PSUM") as ps_o,
    ):
        identB = constp.tile([P, P], dt.bfloat16)
        make_identity(nc, identB)
        identF = constp.tile([P, P], dt.float32)
        make_identity(nc, identF)
        negI = constp.tile([P, P], dt.bfloat16)
        nc.gpsimd.memset(negI, 0.0)
        nc.gpsimd.affine_select(
            out=negI,
            in_=negI,
            compare_op=mybir.AluOpType.not_equal,
            fill=NEGC,
            base=0,
            pattern=[[-1, P]],
            channel_multiplier=1,
        )

        for pr in range(n_pairs):
            heads = (2 * pr, 2 * pr + 1)

            # ---- Q/K: load f32, cast bf16, PE-transpose into [d2, S] slabs ----
            QT2 = qkp.tile([P, S], dt.bfloat16, tag="qt2")
            KT2 = qkp.tile([P, S], dt.bfloat16, tag="kt2")
            for src_ap, slab in ((q_ap, QT2), (k_ap, KT2)):
                for t in range(n_qt):
                    qn = stagep.tile([P, P], dt.bfloat16, tag="qn")
                    for hi, h in enumerate(heads):
                        stf = stagep.tile([P, D], dt.float32, tag="ldstage")
                        nc.sync.dma_start(stf[:], src_ap[h, t * P : (t + 1) * P, :])
                        nc.vector.tensor_copy(qn[:, hi * D : (hi + 1) * D], stf[:])
                    pst = ps_t.tile([P, P], dt.bfloat16, tag="tps")
                    nc.tensor.transpose(pst[:], qn[:], identB[:])
                    nc.vector.tensor_copy(slab[:, t * P : (t + 1) * P], pst[:])

            # ---- V: load f32, cast bf16 into [128, 65] tiles with ones col ----
            v2 = [[None] * n_kch for _ in range(2)]
            for hi, h in enumerate(heads):
                for ki in range(n_kch):
                    stf = stagep.tile([P, D], dt.float32, tag="ldstage")
                    nc.sync.dma_start(stf[:], v_ap[h, ki * P : (ki + 1) * P, :])
                    t2 = vpool.tile([P, D + 1], dt.bfloat16, tag="v2")
                    nc.vector.tensor_copy(t2[:, 0:D], stf[:])
                    nc.vector.memset(t2[:, D : D + 1], 1.0)
                    v2[hi][ki] = t2

            for qc in range(n_qch):
                q0 = qc * qch

                # mask tiles for this q-chunk, natural [q, k] layout, u8->bf16
                mts = [[None] * QS for _ in range(2)]
                for hi, h in enumerate(heads):
                    for qs in range(QS):
                        mt = maskp.tile([P, S], dt.bfloat16, tag="mask")
                        nc.gpsimd.dma_start(
                            mt[:], m_ap[h, q0 + qs * P : q0 + (qs + 1) * P, :]
                        )
                        mts[hi][qs] = mt

                # S^T = K Q^T (paired row-tiled) minus 960*mask^T, then exp
                pts = [[None] * n_kch for _ in range(2)]
                for ki in range(n_kch):
                    k0 = ki * KCH
                    for hi in range(2):
                        st_ = ps_s.tile([P, qch], dt.float32, tag="st")
                        nc.tensor.matmul(
                            st_[:],
                            KT2[hi * D : (hi + 1) * D, k0 : k0 + KCH],
                            QT2[hi * D : (hi + 1) * D, q0 : q0 + qch],
                            start=True,
                            stop=False,
                        )
                        for qs in range(QS):
                            nc.tensor.matmul(
                                st_[:, qs * P : (qs + 1) * P],
                                mts[hi][qs][:, k0 : k0 + KCH],
                                negI[:],
                                start=False,
                                stop=(qs == QS - 1),
                            )
                        pt = ptp.tile([P, qch], dt.bfloat16, tag="pt")
                        nc.scalar.activation(
                            pt[:],
                            st_[:],
                            mybir.ActivationFunctionType.Exp,
                            scale=SCALE,
                        )
                        pts[hi][ki] = pt

                # O^T = [V | 1]^T @ P^T  (M=65: rows 0-63 = O^T, row 64 = l)
                for hi, h in enumerate(heads):
                    po = ps_o.tile([D + 1, qch], dt.float32, tag="po")
                    for ki in range(n_kch):
                        nc.tensor.matmul(
                            po[:],
                            v2[hi][ki][:],
                            pts[hi][ki][:],
                            start=(ki == 0),
                            stop=(ki == n_kch - 1),
                        )
                    osb = opool.tile([D + 1, qch], dt.float32, tag="os")
                    nc.vector.tensor_copy(osb[:], po[:])
                    for ot in range(QS):
                        pst2 = ps_t.tile([P, D + 1], dt.float32, tag="tps")
                        nc.tensor.transpose(
                            pst2[:],
                            osb[:, ot * P : (ot + 1) * P],
                            identF[0 : D + 1, 0 : D + 1],
                        )
                        rc = smallp.tile([P, 1], dt.float32, tag="rc")
                        nc.vector.reciprocal(rc[:], pst2[:, D : D + 1])
                        of = opool.tile([P, D], dt.float32, tag="of")
                        nc.vector.tensor_scalar_mul(of[:], pst2[:, 0:D], rc[:])
                        nc.sync.dma_start(
                            o_ap[h, q0 + ot * P : q0 + (ot + 1) * P, :], of[:]
                        )


def build_program(H=H_PER_CORE, S=S_FULL, qch=512):
    nc = bacc.Bacc()
    q = nc.dram_tensor("q", [H, S, D], mybir.dt.float32, kind="ExternalInput")
    k = nc.dram_tensor("k", [H, S, D], mybir.dt.float32, kind="ExternalInput")
    v = nc.dram_tensor("v", [H, S, D], mybir.dt.float32, kind="ExternalInput")
    m = nc.dram_tensor("m", [H, S, S], mybir.dt.uint8, kind="ExternalInput")
    o = nc.dram_tensor("o", [H, S, D], mybir.dt.float32, kind="ExternalOutput")
    with tile.TileContext(nc) as tc:
        build_attention(tc, o.ap(), q.ap(), k.ap(), v.ap(), m.ap(), H=H, S=S, qch=qch)
    nc.compile()
    return nc


_CACHE = {}
LAST_RESULTS = None


def kernel(queries, keys, values, mask):
    global LAST_RESULTS
    if "nc" not in _CACHE:
        _CACHE["nc"] = build_program()
    nc = _CACHE["nc"]

    queries = np.ascontiguousarray(queries, dtype=np.float32)
    keys = np.ascontiguousarray(keys, dtype=np.float32)
    values = np.ascontiguousarray(values, dtype=np.float32)
    mask_u8 = np.ascontiguousarray(mask).view(np.uint8)

    in_maps = []
    for c in range(N_CORES):
        sl = slice(c * H_PER_CORE, (c + 1) * H_PER_CORE)
        in_maps.append(
            {
                "q": queries[sl],
                "k": keys[sl],
                "v": values[sl],
                "m": mask_u8[sl],
            }
        )

    trace = bool(int(os.environ.get("ATTN_TRACE", "0")))
    res = run_bass_kernel_spmd(
        nc, in_maps, core_ids=list(range(N_CORES)), trace=trace
    )
    LAST_RESULTS = res
    return np.concatenate([r["o"] for r in res.results], axis=0)
